# revision 15
# baseline (speedup 1.0000x reference)
"""Trainium2 Bass kernel for nn_ConditionalNFEncoder.

Computes, for inputs trend/seasonal/residual [B, T]:
  feat_trend    = trend[..., None] * Wt[:, 0] + bt        # [B, T, D]
  feat_seasonal = seasonal[..., None] * Ws[:, 0] + bs     # [B, T, D]
  lp            = MADE-flow log-prob of residual given shifted residual
  out           = concat([feat_trend, feat_seasonal, lp[..., None]], -1)

Sharding: pure data parallel over B across 8 NeuronCores (4 rows each).
Inside a core, tokens are processed in "supertiles" of 1024 tokens: the
flow hidden dim (H=64) is packed twice onto the 128 SBUF partitions
(chunk0 tokens on partitions 0:63, chunk1 on 64:127), free dim = 512
tokens.  The two Linear(1, D) features are computed as K=3 matmuls
(trend/seasonal/ones stationary, [Wt|0 / 0|Ws / bt|bs] moving) directly
in token-major layout, copied PSUM->SBUF, and DMA'd out together with
the log-prob column as [128, 8*1025] tiles.
"""

import numpy as np

import concourse.bass as bass
import concourse.bacc as bacc
import concourse.tile as tile
from concourse import mybir
from concourse._compat import with_exitstack
from concourse.bass_utils import run_bass_kernel_spmd

# Problem constants (hardcoded per contract).
B, T, D, H, S, NBLK = 32, 2048, 512, 64, 3, 2
NCORES = 8
BP = B // NCORES            # batch rows per core = 4
N = BP * T                  # tokens per core = 8192
F = 512                     # flow tile free width (tokens per packed chunk)
ST = 2 * F                  # tokens per supertile = 1024
NST = N // ST               # supertiles per core = 8
ZB = 2                      # supertiles per z-chain batch
NCH = N // 128              # 128-token chunks per core = 64
DOUT = 2 * D + 1            # 1025
LOG_2PI = float(np.log(2.0 * np.pi))

f32 = mybir.dt.float32
AF = mybir.ActivationFunctionType
OP = mybir.AluOpType


def _pack2(v):
    """[H] -> [128] duplicated (chunk0 partitions 0:64, chunk1 64:128)."""
    return np.concatenate([v, v]).astype(np.float32)


def _blockdiag2(m):
    """[H, H] -> [128, 128] block-diagonal with two copies of m."""
    z = np.zeros((2 * H, 2 * H), np.float32)
    z[:H, :H] = m
    z[H:, H:] = m
    return z


def _prep_weights(inp):
    """Host-side packing of the tiny flow / feature weights."""
    w1t = np.zeros((128, S * NBLK * 128), np.float32)
    w2t = np.zeros((128, S * NBLK * 128), np.float32)
    cols = np.zeros((128, 6 + 4 * S * NBLK + S + 1), np.float32)
    wft = np.zeros((128, 4 * S), np.float32)
    for i in range(S):
        cols[:, 30 + i] = float(inp["bf"][i, 0])
    cols[:, 33] = 1e-3
    for i in range(S):
        cols[:, 2 * i] = _pack2(inp["Wc0"][i, :, 0])
        cols[:, 2 * i + 1] = _pack2(inp["bc0"][i] + inp["b_init"][i])
        # wft cols for step i: [u_c0, s_c0, u_c1, s_c1]
        wft[:H, 4 * i + 0] = inp["Wf"][i, 0, :]
        wft[:H, 4 * i + 1] = inp["Wf"][i, 1, :]
        wft[H:, 4 * i + 2] = inp["Wf"][i, 0, :]
        wft[H:, 4 * i + 3] = inp["Wf"][i, 1, :]
        for j in range(NBLK):
            q = i * NBLK + j
            w1t[:, q * 128:(q + 1) * 128] = _blockdiag2(inp["W1"][i, j].T)
            w2t[:, q * 128:(q + 1) * 128] = _blockdiag2(inp["W2"][i, j].T)
            cols[:, 6 + 4 * q + 0] = _pack2(inp["b1"][i, j])
            cols[:, 6 + 4 * q + 1] = _pack2(inp["b2"][i, j])
            cols[:, 6 + 4 * q + 2] = _pack2(inp["Wcb"][i, j, :, 0])
            cols[:, 6 + 4 * q + 3] = _pack2(inp["bcb"][i, j])
    rh = np.zeros((3, 2 * D), np.float32)
    rh[0, :D] = inp["Wt"][:, 0]
    rh[1, D:] = inp["Ws"][:, 0]
    rh[2, :D] = inp["bt"]
    rh[2, D:] = inp["bs"]
    # Merge into two tensors so all constants arrive on two DMA-lane sems:
    # wmm: PE stationary operands [w1t | w2t | wft]   -> [128, 1548]
    # aux: cols + rh (rh padded to 128 partitions)    -> [128, 34 + 1024]
    wmm = np.concatenate([w1t, w2t, wft], axis=1)
    aux = np.zeros((128, cols.shape[1] + 2 * D), np.float32)
    aux[:, :cols.shape[1]] = cols
    aux[:3, cols.shape[1]:] = rh
    return {"wmm": wmm, "aux": aux, "ncols": cols.shape[1]}


def _bcast2_ap(dram_ap_1d, offset, width):
    """DRAM [2*width] slice as a [2, 64, width] AP: two width-chunks, each
    broadcast over 64 partitions (step-0 middle dim).  Zips element-for-
    element with a [128, width] SBUF tile (partition p = 64*c + lane)."""
    s = dram_ap_1d[offset:offset + width]
    return bass.AP(tensor=s.tensor, offset=s.offset,
                   ap=[[width, 2], [0, 64], [1, width]])


@with_exitstack
def _body(ctx, tc, bf, y, tso, cprev, resid, wmm, aux):
    nc = tc.nc

    const = ctx.enter_context(tc.tile_pool(name="const", bufs=1))
    io = ctx.enter_context(tc.tile_pool(name="io", bufs=2))
    flow = ctx.enter_context(tc.tile_pool(name="flow", bufs=2))
    zp = ctx.enter_context(tc.tile_pool(name="zp", bufs=3))
    pmm = ctx.enter_context(tc.tile_pool(name="pmm", bufs=3, space="PSUM"))
    pzt = ctx.enter_context(tc.tile_pool(name="pzt", bufs=1, space="PSUM"))
    pft = ctx.enter_context(tc.tile_pool(name="pft", bufs=2, space="PSUM"))

    # ---- constants into SBUF (two DMAs -> two lane sems) ----
    NCOLS = 6 + 4 * S * NBLK + S + 1  # 34
    wmm_sb = const.tile([128, S * NBLK * 128 * 2 + 4 * S], f32)
    nc.sync.dma_start(out=wmm_sb, in_=wmm)
    aux_sb = const.tile([128, NCOLS + 2 * D], f32)
    nc.sync.dma_start(out=aux_sb, in_=aux)
    x_sb = const.tile([128, NCH], f32)
    nc.sync.dma_start(out=x_sb, in_=resid.rearrange("(g p) -> p g", p=128))

    w1t_sb = wmm_sb[:, 0:S * NBLK * 128]
    w2t_sb = wmm_sb[:, S * NBLK * 128:2 * S * NBLK * 128]
    wft_sb = wmm_sb[:, 2 * S * NBLK * 128:]
    rh_sb = aux_sb[0:3, NCOLS:]

    def col(c):
        return aux_sb[:, c:c + 1]

    # ACT warm-up observer: one single-wait ACT op that makes the ACT
    # engine's vector clock pass the aux DMA lane, so no later ACT
    # instruction (which can encode only ONE sem wait) re-waits it.
    actscr = const.tile([1, 1], f32)
    nc.scalar.copy(actscr, aux_sb[0:1, 0:1])

    for b in range(NST // ZB):
        zt_sb = zp.tile([128, ZB * 4 * S * 4], f32, tag="ztsb")  # [128, 96]

        # ---------- flow for the ZB supertiles of this batch ----------
        for sl in range(ZB):
            s = b * ZB + sl
            cb = flow.tile([128, F], f32, tag="cb")
            nc.gpsimd.dma_start(out=cb, in_=_bcast2_ap(cprev, s * ST, F))
            # DVE-owned copy for the ACT sigmoid: keeps every consumer of a
            # given tile on one engine so no instruction needs >2 sem waits
            # (the ACT instruction encoding supports only 2).
            cb2 = flow.tile([128, F], f32, tag="cb2")
            nc.vector.tensor_copy(cb2, cb)

            zt_ps = pzt.tile([128, 4 * S * 4], f32, tag="ztps")  # [128, 48]
            for i in range(S):
                h = flow.tile([128, F], f32, tag="h")
                nc.vector.tensor_scalar(h, cb, col(2 * i), col(2 * i + 1), OP.mult, OP.add)
                for j in range(NBLK):
                    q = i * NBLK + j
                    r = flow.tile([128, F], f32, tag="r")
                    nc.vector.tensor_scalar_max(r, h, 0.0)
                    p1 = pmm.tile([128, F], f32, tag="pmm")
                    nc.tensor.matmul(p1, w1t_sb[:, q * 128:(q + 1) * 128], r,
                                     start=True, stop=True)
                    r1 = flow.tile([128, F], f32, tag="r1")
                    nc.scalar.activation(r1, p1, AF.Relu, bias=col(6 + 4 * q + 0))
                    p2 = pmm.tile([128, F], f32, tag="pmm")
                    nc.tensor.matmul(p2, w2t_sb[:, q * 128:(q + 1) * 128], r1,
                                     start=True, stop=True)
                    sg = flow.tile([128, F], f32, tag="sg")
                    nc.scalar.activation(sg, cb2, AF.Sigmoid,
                                         bias=col(6 + 4 * q + 3), scale=col(6 + 4 * q + 2))
                    t2 = flow.tile([128, F], f32, tag="t2")
                    nc.vector.tensor_scalar_add(t2, p2, col(6 + 4 * q + 1))
                    m = flow.tile([128, F], f32, tag="m")
                    nc.vector.tensor_tensor(m, t2, sg, OP.mult)
                    h2 = flow.tile([128, F], f32, tag="h")
                    nc.vector.tensor_tensor(h2, h, m, OP.add)
                    h = h2
                r2 = flow.tile([128, F], f32, tag="r")
                nc.vector.tensor_scalar_max(r2, h, 0.0)
                # transpose (uscale, shift) to token-major via tiny matmuls:
                # out[p_tok, 4] = r2[:, j2-chunk].T @ wft_i
                for j2 in range(4):
                    c0 = 4 * (S * j2 + i)
                    nc.tensor.matmul(zt_ps[:, c0:c0 + 4],
                                     r2[:, 128 * j2:128 * (j2 + 1)],
                                     wft_sb[:, 4 * i:4 * i + 4],
                                     start=True, stop=True)
            nc.vector.tensor_copy(zt_sb[:, sl * 48:(sl + 1) * 48], zt_ps)

        # ---------- z-chain for this batch (token-major, [128, ZB, 2, 4]) ----------
        # zt_sb col = sl*48 + j2*12 + i*4 + c*2 + t
        V = zt_sb.rearrange("p (s j i c t) -> p t i s c j", s=ZB, j=4, i=S, c=2, t=2)
        zsh = [128, ZB, 2, 4]
        z = zp.tile(zsh, f32, tag="z")
        xv = x_sb[:, b * ZB * 8:(b + 1) * ZB * 8].rearrange(
            "p (s c j) -> p s c j", s=ZB, c=2, j=4)
        nc.vector.tensor_copy(z, xv)
        ld = None
        for i in range(S):
            u_v = V[:, 0, i]
            s_v = V[:, 1, i]
            # softplus(u + bf0) = ln(1 + exp(u + bf0)) — Softplus has no ACT
            # table set on this toolchain; Exp and Ln share one set.
            ex = zp.tile(zsh, f32, tag="ex")
            nc.scalar.activation(ex, u_v, AF.Exp, bias=col(30 + i))
            sp = zp.tile(zsh, f32, tag="sp")
            nc.scalar.activation(sp, ex, AF.Ln, bias=1.0)
            sc = zp.tile(zsh, f32, tag="sc")
            nc.vector.tensor_scalar_add(sc, sp, 1e-3)
            ldi = zp.tile(zsh, f32, tag="ldi")
            nc.scalar.activation(ldi, sp, AF.Ln, bias=col(33))
            if ld is None:
                ld = ldi
            else:
                ld2 = zp.tile(zsh, f32, tag="ld")
                nc.vector.tensor_tensor(ld2, ld, ldi, OP.add)
                ld = ld2
            z2 = zp.tile(zsh, f32, tag="z")
            nc.vector.tensor_tensor(z2, z, sc, OP.mult)
            sh = zp.tile(zsh, f32, tag="sh")
            nc.vector.tensor_scalar_add(sh, s_v, float(bf[i, 1]))
            z3 = zp.tile(zsh, f32, tag="z")
            nc.vector.tensor_tensor(z3, z2, sh, OP.add)
            z = z3
        zz = zp.tile(zsh, f32, tag="zz")
        nc.vector.tensor_tensor(zz, z, z, OP.mult)
        lp1 = zp.tile(zsh, f32, tag="lp1")
        nc.vector.tensor_scalar(lp1, zz, -0.5, -0.5 * LOG_2PI, OP.mult, OP.add)
        lp = zp.tile(zsh, f32, tag="lp")
        nc.vector.tensor_tensor(lp, lp1, ld, OP.add)

        # ---------- features + output assembly for the ZB supertiles ----------
        for sl in range(ZB):
            s = b * ZB + sl
            outt = io.tile([128, 8 * DOUT], f32, tag="outt")
            nc.scalar.copy(outt[0:1, 0:1], actscr)
            outr = outt.rearrange("p (k c) -> p k c", c=DOUT)
            tso_g = io.tile([3, ST], f32, tag="tsog")
            nc.sync.dma_start(out=tso_g, in_=tso[:, s * ST:(s + 1) * ST])
            for k in range(8):
                fp = pft.tile([128, 2 * D], f32, tag="fp")
                lhs = tso_g[:, k * 128:(k + 1) * 128]
                nc.tensor.matmul(fp[:, 0:D], lhs, rh_sb[:, 0:D], start=True, stop=True)
                nc.tensor.matmul(fp[:, D:2 * D], lhs, rh_sb[:, D:2 * D],
                                 start=True, stop=True)
                if k % 2 == 0:
                    nc.scalar.copy(outr[:, k, 0:2 * D], fp)
                else:
                    nc.vector.tensor_copy(outr[:, k, 0:2 * D], fp)
            lpv = outt.rearrange("p (c j cc) -> p c j cc", c=2, j=4)[:, :, :, 2 * D]
            nc.vector.tensor_copy(lpv, lp[:, sl])
            ydst = y.rearrange("(s k p) c -> s p k c", p=128, k=8)[s]
            nc.sync.dma_start(out=ydst, in_=outt)


def _build_module(bf):
    nc = bacc.Bacc("TRN2", target_bir_lowering=False, debug=False,
                   enable_asserts=False, num_devices=NCORES)
    y = nc.dram_tensor("y", [N, DOUT], f32, kind="ExternalOutput").ap()
    tso = nc.dram_tensor("tso", [3, N], f32, kind="ExternalInput").ap()
    cprev = nc.dram_tensor("cprev", [N], f32, kind="ExternalInput").ap()
    resid = nc.dram_tensor("resid", [N], f32, kind="ExternalInput").ap()
    wmm = nc.dram_tensor("wmm", [128, S * NBLK * 128 * 2 + 4 * S], f32, kind="ExternalInput").ap()
    aux = nc.dram_tensor("aux", [128, 6 + 4 * S * NBLK + S + 1 + 2 * D], f32, kind="ExternalInput").ap()
    with tile.TileContext(nc) as tc:
        _body(tc, bf, y, tso, cprev, resid, wmm, aux)
    nc.compile()
    return nc


def _run(inputs, trace=False):
    wp = _prep_weights(inputs)
    bf = np.asarray(inputs["bf"], np.float32)
    nc = _build_module(bf)

    trend = np.asarray(inputs["trend"], np.float32)
    seasonal = np.asarray(inputs["seasonal"], np.float32)
    residual = np.asarray(inputs["residual"], np.float32)
    prev = np.concatenate([np.zeros_like(residual[:, :1]), residual[:, :-1]], axis=1)

    in_maps = []
    for c in range(NCORES):
        sl = slice(c * BP, (c + 1) * BP)
        tso = np.empty((3, N), np.float32)
        tso[0] = trend[sl].reshape(-1)
        tso[1] = seasonal[sl].reshape(-1)
        tso[2] = 1.0
        in_maps.append({
            "tso": tso,
            "cprev": np.ascontiguousarray(prev[sl].reshape(-1)),
            "resid": np.ascontiguousarray(residual[sl].reshape(-1)),
            "wmm": wp["wmm"], "aux": wp["aux"],
        })

    res = run_bass_kernel_spmd(nc, in_maps, core_ids=list(range(NCORES)),
                               trace=trace)
    out = np.concatenate(
        [r["y"].reshape(BP, T, DOUT) for r in res.results], axis=0)
    return out, res


def kernel(**inputs):
    out, _ = _run(inputs, trace=False)
    return out


# revision 16
# speedup vs baseline: 1.5931x; 1.5931x over previous
"""Trainium2 Bass kernel for nn_ConditionalNFEncoder.

Computes, for inputs trend/seasonal/residual [B, T]:
  feat_trend    = trend[..., None] * Wt[:, 0] + bt        # [B, T, D]
  feat_seasonal = seasonal[..., None] * Ws[:, 0] + bs     # [B, T, D]
  lp            = MADE-flow log-prob of residual given shifted residual
  out           = concat([feat_trend, feat_seasonal, lp[..., None]], -1)

Sharding: pure data parallel over B across 8 NeuronCores (4 rows each).
Inside a core, tokens are processed in "supertiles" of 1024 tokens: the
flow hidden dim (H=64) is packed twice onto the 128 SBUF partitions
(chunk0 tokens on partitions 0:63, chunk1 on 64:127), free dim = 512
tokens.  The two Linear(1, D) features are computed as K=3 matmuls
(trend/seasonal/ones stationary, [Wt|0 / 0|Ws / bt|bs] moving) directly
in token-major layout, copied PSUM->SBUF, and DMA'd out together with
the log-prob column as [128, 8*1025] tiles.
"""

import numpy as np
import ml_dtypes

import concourse.bass as bass
import concourse.bacc as bacc
import concourse.tile as tile
from concourse import mybir
from concourse._compat import with_exitstack
from concourse.bass_utils import run_bass_kernel_spmd

# Problem constants (hardcoded per contract).
B, T, D, H, S, NBLK = 32, 2048, 512, 64, 3, 2
NCORES = 8
BP = B // NCORES            # batch rows per core = 4
N = BP * T                  # tokens per core = 8192
F = 512                     # flow tile free width (tokens per packed chunk)
ST = 2 * F                  # tokens per supertile = 1024
NST = N // ST               # supertiles per core = 8
ZB = 4                      # supertiles per z-chain batch
NCH = N // 128              # 128-token chunks per core = 64
DOUT = 2 * D + 1            # 1025
LOG_2PI = float(np.log(2.0 * np.pi))

f32 = mybir.dt.float32
bf16 = mybir.dt.bfloat16
AF = mybir.ActivationFunctionType
OP = mybir.AluOpType


def _pack2(v):
    """[H] -> [128] duplicated (chunk0 partitions 0:64, chunk1 64:128)."""
    return np.concatenate([v, v]).astype(np.float32)


def _blockdiag2(m):
    """[H, H] -> [128, 128] block-diagonal with two copies of m."""
    z = np.zeros((2 * H, 2 * H), np.float32)
    z[:H, :H] = m
    z[H:, H:] = m
    return z


def _prep_weights(inp):
    """Host-side packing of the tiny flow / feature weights."""
    w1t = np.zeros((128, S * NBLK * 128), np.float32)
    w2t = np.zeros((128, S * NBLK * 128), np.float32)
    cols = np.zeros((128, 6 + 4 * S * NBLK + S + 1), np.float32)
    wft = np.zeros((128, 4 * S), np.float32)
    for i in range(S):
        cols[:, 30 + i] = float(inp["bf"][i, 0])
    cols[:, 33] = 1e-3
    for i in range(S):
        cols[:, 2 * i] = _pack2(inp["Wc0"][i, :, 0])
        cols[:, 2 * i + 1] = _pack2(inp["bc0"][i] + inp["b_init"][i])
        # wft cols for step i: [u_c0, s_c0, u_c1, s_c1]
        wft[:H, 4 * i + 0] = inp["Wf"][i, 0, :]
        wft[:H, 4 * i + 1] = inp["Wf"][i, 1, :]
        wft[H:, 4 * i + 2] = inp["Wf"][i, 0, :]
        wft[H:, 4 * i + 3] = inp["Wf"][i, 1, :]
        for j in range(NBLK):
            q = i * NBLK + j
            w1t[:, q * 128:(q + 1) * 128] = _blockdiag2(inp["W1"][i, j].T)
            w2t[:, q * 128:(q + 1) * 128] = _blockdiag2(inp["W2"][i, j].T)
            cols[:, 6 + 4 * q + 0] = _pack2(inp["b1"][i, j])
            cols[:, 6 + 4 * q + 1] = _pack2(inp["b2"][i, j])
            cols[:, 6 + 4 * q + 2] = _pack2(inp["Wcb"][i, j, :, 0])
            cols[:, 6 + 4 * q + 3] = _pack2(inp["bcb"][i, j])
    rh = np.zeros((3, 2 * D), np.float32)
    rh[0, :D] = inp["Wt"][:, 0]
    rh[1, D:] = inp["Ws"][:, 0]
    rh[2, :D] = inp["bt"]
    rh[2, D:] = inp["bs"]
    # Merge into two tensors so all constants arrive on two DMA-lane sems:
    # wmm (bf16): PE operands [w1t | w2t | wft | rh(padded)] -> [128, 2572]
    # aux (f32):  per-partition scalar columns               -> [128, 34]
    rhp = np.zeros((128, 2 * D), np.float32)
    rhp[:3] = rh
    wmm = np.concatenate([w1t, w2t, wft, rhp], axis=1).astype(ml_dtypes.bfloat16)
    return {"wmm": wmm, "aux": cols, "ncols": cols.shape[1]}


def _bcast2_ap(dram_ap_1d, offset, width):
    """DRAM [2*width] slice as a [2, 64, width] AP: two width-chunks, each
    broadcast over 64 partitions (step-0 middle dim).  Zips element-for-
    element with a [128, width] SBUF tile (partition p = 64*c + lane)."""
    s = dram_ap_1d[offset:offset + width]
    return bass.AP(tensor=s.tensor, offset=s.offset,
                   ap=[[width, 2], [0, 64], [1, width]])


@with_exitstack
def _body(ctx, tc, bf, y, tso, cprev, resid, wmm, aux):
    nc = tc.nc

    const = ctx.enter_context(tc.tile_pool(name="const", bufs=1))
    io = ctx.enter_context(tc.tile_pool(name="io", bufs=2))
    flow = ctx.enter_context(tc.tile_pool(name="flow", bufs=2))
    zp = ctx.enter_context(tc.tile_pool(name="zp", bufs=3))
    pmm = ctx.enter_context(tc.tile_pool(name="pmm", bufs=3, space="PSUM"))
    pzt = ctx.enter_context(tc.tile_pool(name="pzt", bufs=1, space="PSUM"))
    pft = ctx.enter_context(tc.tile_pool(name="pft", bufs=2, space="PSUM"))

    # ---- constants into SBUF (two DMAs -> two lane sems) ----
    NCOLS = 6 + 4 * S * NBLK + S + 1  # 34
    WMMW = S * NBLK * 128 * 2 + 4 * S + 2 * D  # 2572 bf16 cols
    wmm_sb = const.tile([128, WMMW], bf16)
    nc.sync.dma_start(out=wmm_sb, in_=wmm)
    aux_sb = const.tile([128, NCOLS], f32)
    nc.sync.dma_start(out=aux_sb, in_=aux)
    x_sb = const.tile([128, NCH], f32)
    nc.sync.dma_start(out=x_sb, in_=resid.rearrange("(g p) -> p g", p=128))

    w1t_sb = wmm_sb[:, 0:S * NBLK * 128]
    w2t_sb = wmm_sb[:, S * NBLK * 128:2 * S * NBLK * 128]
    wft_sb = wmm_sb[:, 2 * S * NBLK * 128:2 * S * NBLK * 128 + 4 * S]
    rh_sb = wmm_sb[0:3, 2 * S * NBLK * 128 + 4 * S:]

    def col(c):
        return aux_sb[:, c:c + 1]

    # ACT warm-up observer: one single-wait ACT op that makes the ACT
    # engine's vector clock pass the aux DMA lane, so no later ACT
    # instruction (which can encode only ONE sem wait) re-waits it.
    actscr = const.tile([1, 1], f32)
    nc.scalar.copy(actscr, aux_sb[0:1, 0:1])

    for b in range(NST // ZB):
        zt_sb = zp.tile([128, ZB * 4 * S * 4], f32, tag="ztsb")  # [128, 96]

        # ---------- flow for the ZB supertiles of this batch ----------
        for sl in range(ZB):
            s = b * ZB + sl
            cb = flow.tile([128, F], f32, tag="cb")
            nc.gpsimd.dma_start(out=cb, in_=_bcast2_ap(cprev, s * ST, F))
            # DVE-owned copy for the ACT sigmoid: keeps every consumer of a
            # given tile on one engine so no instruction needs >2 sem waits
            # (the ACT instruction encoding supports only 2).
            cb2 = flow.tile([128, F], f32, tag="cb2")
            nc.vector.tensor_copy(cb2, cb)

            zt_ps = pzt.tile([128, 4 * S * 4], f32, tag="ztps")  # [128, 48]
            for i in range(S):
                h = flow.tile([128, F], f32, tag="h")
                nc.vector.tensor_scalar(h, cb, col(2 * i), col(2 * i + 1), OP.mult, OP.add)
                for j in range(NBLK):
                    q = i * NBLK + j
                    r = flow.tile([128, F], bf16, tag="r")
                    nc.vector.tensor_scalar_max(r, h, 0.0)
                    p1 = pmm.tile([128, F], f32, tag="pmm")
                    nc.tensor.matmul(p1, w1t_sb[:, q * 128:(q + 1) * 128], r,
                                     start=True, stop=True)
                    r1 = flow.tile([128, F], bf16, tag="r1")
                    nc.scalar.activation(r1, p1, AF.Relu, bias=col(6 + 4 * q + 0))
                    p2 = pmm.tile([128, F], f32, tag="pmm")
                    nc.tensor.matmul(p2, w2t_sb[:, q * 128:(q + 1) * 128], r1,
                                     start=True, stop=True)
                    sg = flow.tile([128, F], f32, tag="sg")
                    nc.scalar.activation(sg, cb2, AF.Sigmoid,
                                         bias=col(6 + 4 * q + 3), scale=col(6 + 4 * q + 2))
                    t2 = flow.tile([128, F], f32, tag="t2")
                    nc.vector.tensor_scalar_add(t2, p2, col(6 + 4 * q + 1))
                    m = flow.tile([128, F], f32, tag="m")
                    nc.vector.tensor_tensor(m, t2, sg, OP.mult)
                    h2 = flow.tile([128, F], f32, tag="h")
                    nc.vector.tensor_tensor(h2, h, m, OP.add)
                    h = h2
                r2 = flow.tile([128, F], bf16, tag="r")
                nc.vector.tensor_scalar_max(r2, h, 0.0)
                # transpose (uscale, shift) to token-major via tiny matmuls:
                # out[p_tok, 4] = r2[:, j2-chunk].T @ wft_i
                for j2 in range(4):
                    c0 = 4 * (S * j2 + i)
                    nc.tensor.matmul(zt_ps[:, c0:c0 + 4],
                                     r2[:, 128 * j2:128 * (j2 + 1)],
                                     wft_sb[:, 4 * i:4 * i + 4],
                                     start=True, stop=True)
            nc.vector.tensor_copy(zt_sb[:, sl * 48:(sl + 1) * 48], zt_ps)

        # ---------- z-chain for this batch (token-major, [128, ZB, 2, 4]) ----------
        # zt_sb col = sl*48 + j2*12 + i*4 + c*2 + t
        V = zt_sb.rearrange("p (s j i c t) -> p t i s c j", s=ZB, j=4, i=S, c=2, t=2)
        zsh = [128, ZB, 2, 4]
        z = zp.tile(zsh, f32, tag="z")
        xv = x_sb[:, b * ZB * 8:(b + 1) * ZB * 8].rearrange(
            "p (s c j) -> p s c j", s=ZB, c=2, j=4)
        nc.vector.tensor_copy(z, xv)
        ld = None
        for i in range(S):
            u_v = V[:, 0, i]
            s_v = V[:, 1, i]
            # softplus(u + bf0) = ln(1 + exp(u + bf0)) — Softplus has no ACT
            # table set on this toolchain; Exp and Ln share one set.
            ex = zp.tile(zsh, f32, tag="ex")
            nc.scalar.activation(ex, u_v, AF.Exp, bias=col(30 + i))
            sp = zp.tile(zsh, f32, tag="sp")
            nc.scalar.activation(sp, ex, AF.Ln, bias=1.0)
            sc = zp.tile(zsh, f32, tag="sc")
            nc.vector.tensor_scalar_add(sc, sp, 1e-3)
            ldi = zp.tile(zsh, f32, tag="ldi")
            nc.scalar.activation(ldi, sp, AF.Ln, bias=col(33))
            if ld is None:
                ld = ldi
            else:
                ld2 = zp.tile(zsh, f32, tag="ld")
                nc.vector.tensor_tensor(ld2, ld, ldi, OP.add)
                ld = ld2
            z2 = zp.tile(zsh, f32, tag="z")
            nc.vector.tensor_tensor(z2, z, sc, OP.mult)
            sh = zp.tile(zsh, f32, tag="sh")
            nc.vector.tensor_scalar_add(sh, s_v, float(bf[i, 1]))
            z3 = zp.tile(zsh, f32, tag="z")
            nc.vector.tensor_tensor(z3, z2, sh, OP.add)
            z = z3
        zz = zp.tile(zsh, f32, tag="zz")
        nc.vector.tensor_tensor(zz, z, z, OP.mult)
        lp1 = zp.tile(zsh, f32, tag="lp1")
        nc.vector.tensor_scalar(lp1, zz, -0.5, -0.5 * LOG_2PI, OP.mult, OP.add)
        lp = zp.tile(zsh, f32, tag="lp")
        nc.vector.tensor_tensor(lp, lp1, ld, OP.add)

        # ---------- features + output assembly for the ZB supertiles ----------
        for sl in range(ZB):
            s = b * ZB + sl
            outt = io.tile([128, 8 * DOUT], f32, tag="outt")
            nc.scalar.copy(outt[0:1, 0:1], actscr)
            outr = outt.rearrange("p (k c) -> p k c", c=DOUT)
            tso_g = io.tile([3, ST], bf16, tag="tsog")
            nc.sync.dma_start(out=tso_g, in_=tso[:, s * ST:(s + 1) * ST])
            for k in range(8):
                fp = pft.tile([128, 2 * D], f32, tag="fp")
                lhs = tso_g[:, k * 128:(k + 1) * 128]
                nc.tensor.matmul(fp[:, 0:D], lhs, rh_sb[:, 0:D], start=True, stop=True)
                nc.tensor.matmul(fp[:, D:2 * D], lhs, rh_sb[:, D:2 * D],
                                 start=True, stop=True)
                if k % 2 == 0:
                    nc.scalar.copy(outr[:, k, 0:2 * D], fp)
                else:
                    nc.vector.tensor_copy(outr[:, k, 0:2 * D], fp)
            lpv = outt.rearrange("p (c j cc) -> p c j cc", c=2, j=4)[:, :, :, 2 * D]
            nc.vector.tensor_copy(lpv, lp[:, sl])
            ydst = y.rearrange("(s k p) c -> s p k c", p=128, k=8)[s]
            nc.sync.dma_start(out=ydst, in_=outt)


def _build_module(bf):
    nc = bacc.Bacc("TRN2", target_bir_lowering=False, debug=False,
                   enable_asserts=False, num_devices=NCORES)
    y = nc.dram_tensor("y", [N, DOUT], f32, kind="ExternalOutput").ap()
    tso = nc.dram_tensor("tso", [3, N], bf16, kind="ExternalInput").ap()
    cprev = nc.dram_tensor("cprev", [N], f32, kind="ExternalInput").ap()
    resid = nc.dram_tensor("resid", [N], f32, kind="ExternalInput").ap()
    wmm = nc.dram_tensor("wmm", [128, S * NBLK * 128 * 2 + 4 * S + 2 * D], bf16, kind="ExternalInput").ap()
    aux = nc.dram_tensor("aux", [128, 6 + 4 * S * NBLK + S + 1], f32, kind="ExternalInput").ap()
    with tile.TileContext(nc) as tc:
        _body(tc, bf, y, tso, cprev, resid, wmm, aux)
    nc.compile()
    return nc


def _run(inputs, trace=False):
    wp = _prep_weights(inputs)
    bf = np.asarray(inputs["bf"], np.float32)
    nc = _build_module(bf)

    trend = np.asarray(inputs["trend"], np.float32)
    seasonal = np.asarray(inputs["seasonal"], np.float32)
    residual = np.asarray(inputs["residual"], np.float32)
    prev = np.concatenate([np.zeros_like(residual[:, :1]), residual[:, :-1]], axis=1)

    in_maps = []
    for c in range(NCORES):
        sl = slice(c * BP, (c + 1) * BP)
        tso = np.empty((3, N), ml_dtypes.bfloat16)
        tso[0] = trend[sl].reshape(-1).astype(ml_dtypes.bfloat16)
        tso[1] = seasonal[sl].reshape(-1).astype(ml_dtypes.bfloat16)
        tso[2] = 1.0
        in_maps.append({
            "tso": tso,
            "cprev": np.ascontiguousarray(prev[sl].reshape(-1)),
            "resid": np.ascontiguousarray(residual[sl].reshape(-1)),
            "wmm": wp["wmm"], "aux": wp["aux"],
        })

    res = run_bass_kernel_spmd(nc, in_maps, core_ids=list(range(NCORES)),
                               trace=trace)
    out = np.concatenate(
        [r["y"].reshape(BP, T, DOUT) for r in res.results], axis=0)
    return out, res


def kernel(**inputs):
    out, _ = _run(inputs, trace=False)
    return out


# revision 17
# speedup vs baseline: 1.8050x; 1.1330x over previous
"""Trainium2 Bass kernel for nn_ConditionalNFEncoder.

Computes, for inputs trend/seasonal/residual [B, T]:
  feat_trend    = trend[..., None] * Wt[:, 0] + bt        # [B, T, D]
  feat_seasonal = seasonal[..., None] * Ws[:, 0] + bs     # [B, T, D]
  lp            = MADE-flow log-prob of residual given shifted residual
  out           = concat([feat_trend, feat_seasonal, lp[..., None]], -1)

Sharding: pure data parallel over B across 8 NeuronCores (4 rows each).
Inside a core, tokens are processed in "supertiles" of 1024 tokens: the
flow hidden dim (H=64) is packed twice onto the 128 SBUF partitions
(chunk0 tokens on partitions 0:63, chunk1 on 64:127), free dim = 512
tokens.  The two Linear(1, D) features are computed as K=3 matmuls
(trend/seasonal/ones stationary, [Wt|0 / 0|Ws / bt|bs] moving) directly
in token-major layout, copied PSUM->SBUF, and DMA'd out together with
the log-prob column as [128, 8*1025] tiles.
"""

import numpy as np
import ml_dtypes

import concourse.bass as bass
import concourse.bacc as bacc
import concourse.tile as tile
from concourse import mybir
from concourse._compat import with_exitstack
from concourse.bass_utils import run_bass_kernel_spmd

# Problem constants (hardcoded per contract).
B, T, D, H, S, NBLK = 32, 2048, 512, 64, 3, 2
NCORES = 8
BP = B // NCORES            # batch rows per core = 4
N = BP * T                  # tokens per core = 8192
F = 512                     # flow tile free width (tokens per packed chunk)
ST = 2 * F                  # tokens per supertile = 1024
NST = N // ST               # supertiles per core = 8
ZB = 4                      # supertiles per z-chain batch
NCH = N // 128              # 128-token chunks per core = 64
DOUT = 2 * D + 1            # 1025
LOG_2PI = float(np.log(2.0 * np.pi))

f32 = mybir.dt.float32
bf16 = mybir.dt.bfloat16
AF = mybir.ActivationFunctionType
OP = mybir.AluOpType


def _pack2(v):
    """[H] -> [128] duplicated (chunk0 partitions 0:64, chunk1 64:128)."""
    return np.concatenate([v, v]).astype(np.float32)


def _blockdiag2(m):
    """[H, H] -> [128, 128] block-diagonal with two copies of m."""
    z = np.zeros((2 * H, 2 * H), np.float32)
    z[:H, :H] = m
    z[H:, H:] = m
    return z


def _prep_weights(inp):
    """Host-side packing of the tiny flow / feature weights."""
    w1t = np.zeros((128, S * NBLK * 128), np.float32)
    w2t = np.zeros((128, S * NBLK * 128), np.float32)
    cols = np.zeros((128, 6 + 4 * S * NBLK + S + 1), np.float32)
    wft = np.zeros((128, 4 * S), np.float32)
    for i in range(S):
        cols[:, 30 + i] = float(inp["bf"][i, 0])
    cols[:, 33] = 1e-3
    for i in range(S):
        cols[:, 2 * i] = _pack2(inp["Wc0"][i, :, 0])
        cols[:, 2 * i + 1] = _pack2(inp["bc0"][i] + inp["b_init"][i])
        # wft cols for step i: [u_c0, s_c0, u_c1, s_c1]
        wft[:H, 4 * i + 0] = inp["Wf"][i, 0, :]
        wft[:H, 4 * i + 1] = inp["Wf"][i, 1, :]
        wft[H:, 4 * i + 2] = inp["Wf"][i, 0, :]
        wft[H:, 4 * i + 3] = inp["Wf"][i, 1, :]
        for j in range(NBLK):
            q = i * NBLK + j
            w1t[:, q * 128:(q + 1) * 128] = _blockdiag2(inp["W1"][i, j].T)
            w2t[:, q * 128:(q + 1) * 128] = _blockdiag2(inp["W2"][i, j].T)
            cols[:, 6 + 4 * q + 0] = _pack2(inp["b1"][i, j])
            cols[:, 6 + 4 * q + 1] = _pack2(inp["b2"][i, j])
            cols[:, 6 + 4 * q + 2] = _pack2(inp["Wcb"][i, j, :, 0])
            cols[:, 6 + 4 * q + 3] = _pack2(inp["bcb"][i, j])
    rh = np.zeros((3, 2 * D), np.float32)
    rh[0, :D] = inp["Wt"][:, 0]
    rh[1, D:] = inp["Ws"][:, 0]
    rh[2, :D] = inp["bt"]
    rh[2, D:] = inp["bs"]
    # Merge into two tensors so all constants arrive on two DMA-lane sems:
    # wmm (bf16): PE operands [w1t | w2t | wft | rh(padded)] -> [128, 2572]
    # aux (f32):  per-partition scalar columns               -> [128, 34]
    rhp = np.zeros((128, 2 * D), np.float32)
    rhp[:3] = rh
    wmm = np.concatenate([w1t, w2t, wft, rhp], axis=1).astype(ml_dtypes.bfloat16)
    return {"wmm": wmm, "aux": cols, "ncols": cols.shape[1]}


def _bcast2_ap(dram_ap_1d, offset, width):
    """DRAM [2*width] slice as a [2, 64, width] AP: two width-chunks, each
    broadcast over 64 partitions (step-0 middle dim).  Zips element-for-
    element with a [128, width] SBUF tile (partition p = 64*c + lane)."""
    s = dram_ap_1d[offset:offset + width]
    return bass.AP(tensor=s.tensor, offset=s.offset,
                   ap=[[width, 2], [0, 64], [1, width]])


@with_exitstack
def _body(ctx, tc, bf, y, tso, cprev, resid, wmm, aux):
    nc = tc.nc

    const = ctx.enter_context(tc.tile_pool(name="const", bufs=1))
    io = ctx.enter_context(tc.tile_pool(name="io", bufs=2))
    flow = ctx.enter_context(tc.tile_pool(name="flow", bufs=3))
    zp = ctx.enter_context(tc.tile_pool(name="zp", bufs=3))
    pmm = ctx.enter_context(tc.tile_pool(name="pmm", bufs=2, space="PSUM"))
    pzt = ctx.enter_context(tc.tile_pool(name="pzt", bufs=2, space="PSUM"))
    pft = ctx.enter_context(tc.tile_pool(name="pft", bufs=2, space="PSUM"))

    # ---- constants into SBUF (two DMAs -> two lane sems) ----
    NCOLS = 6 + 4 * S * NBLK + S + 1  # 34
    WMMW = S * NBLK * 128 * 2 + 4 * S + 2 * D  # 2572 bf16 cols
    wmm_sb = const.tile([128, WMMW], bf16)
    nc.sync.dma_start(out=wmm_sb, in_=wmm)
    aux_sb = const.tile([128, NCOLS], f32)
    nc.sync.dma_start(out=aux_sb, in_=aux)
    x_sb = const.tile([128, NCH], f32)
    nc.sync.dma_start(out=x_sb, in_=resid.rearrange("(g p) -> p g", p=128))

    w1t_sb = wmm_sb[:, 0:S * NBLK * 128]
    w2t_sb = wmm_sb[:, S * NBLK * 128:2 * S * NBLK * 128]
    wft_sb = wmm_sb[:, 2 * S * NBLK * 128:2 * S * NBLK * 128 + 4 * S]
    rh_sb = wmm_sb[0:3, 2 * S * NBLK * 128 + 4 * S:]

    def col(c):
        return aux_sb[:, c:c + 1]

    # ACT warm-up observer: one single-wait ACT op that makes the ACT
    # engine's vector clock pass the aux DMA lane, so no later ACT
    # instruction (which can encode only ONE sem wait) re-waits it.
    actscr = const.tile([1, 1], f32)
    nc.scalar.copy(actscr, aux_sb[0:1, 0:1])

    for b in range(NST // ZB):
        zt_sb = zp.tile([128, ZB * 4 * S * 4], f32, tag="ztsb")  # [128, 96]

        # ---------- flow for the ZB supertiles of this batch ----------
        for sl in range(ZB):
            s = b * ZB + sl
            cb = flow.tile([128, F], bf16, tag="cb")
            nc.gpsimd.dma_start(out=cb, in_=_bcast2_ap(cprev, s * ST, F))
            # DVE-owned copy for the ACT sigmoid: keeps every consumer of a
            # given tile on one engine so no instruction needs >2 sem waits
            # (the ACT instruction encoding supports only 2).
            cb2 = flow.tile([128, F], bf16, tag="cb2")
            nc.vector.tensor_copy(cb2, cb)

            zt_ps = pzt.tile([128, 4 * S * 4], f32, tag="ztps")  # [128, 48]
            for i in range(S):
                h = flow.tile([128, F], bf16, tag="h")
                nc.vector.tensor_scalar(h, cb, col(2 * i), col(2 * i + 1), OP.mult, OP.add)
                for j in range(NBLK):
                    q = i * NBLK + j
                    r = flow.tile([128, F], bf16, tag="r")
                    nc.vector.tensor_scalar_max(r, h, 0.0)
                    p1 = pmm.tile([128, F], f32, tag="pmm")
                    nc.tensor.matmul(p1, w1t_sb[:, q * 128:(q + 1) * 128], r,
                                     start=True, stop=True)
                    r1 = flow.tile([128, F], bf16, tag="r1")
                    nc.scalar.activation(r1, p1, AF.Relu, bias=col(6 + 4 * q + 0))
                    p2 = pmm.tile([128, F], f32, tag="pmm")
                    nc.tensor.matmul(p2, w2t_sb[:, q * 128:(q + 1) * 128], r1,
                                     start=True, stop=True)
                    sg = flow.tile([128, F], bf16, tag="sg")
                    nc.scalar.activation(sg, cb2, AF.Sigmoid,
                                         bias=col(6 + 4 * q + 3), scale=col(6 + 4 * q + 2))
                    t2 = flow.tile([128, F], bf16, tag="t2")
                    nc.vector.tensor_scalar_add(t2, p2, col(6 + 4 * q + 1))
                    m = flow.tile([128, F], bf16, tag="m")
                    nc.vector.tensor_tensor(m, t2, sg, OP.mult)
                    h2 = flow.tile([128, F], bf16, tag="h")
                    nc.vector.tensor_tensor(h2, h, m, OP.add)
                    h = h2
                r2 = flow.tile([128, F], bf16, tag="r")
                nc.vector.tensor_scalar_max(r2, h, 0.0)
                # transpose (uscale, shift) to token-major via tiny matmuls:
                # out[p_tok, 4] = r2[:, j2-chunk].T @ wft_i
                for j2 in range(4):
                    c0 = 4 * (S * j2 + i)
                    nc.tensor.matmul(zt_ps[:, c0:c0 + 4],
                                     r2[:, 128 * j2:128 * (j2 + 1)],
                                     wft_sb[:, 4 * i:4 * i + 4],
                                     start=True, stop=True)
            nc.vector.tensor_copy(zt_sb[:, sl * 48:(sl + 1) * 48], zt_ps)

        # ---------- z-chain for this batch (token-major, [128, ZB, 2, 4]) ----------
        # zt_sb col = sl*48 + j2*12 + i*4 + c*2 + t
        V = zt_sb.rearrange("p (s j i c t) -> p t i s c j", s=ZB, j=4, i=S, c=2, t=2)
        zsh = [128, ZB, 2, 4]
        z = zp.tile(zsh, f32, tag="z")
        xv = x_sb[:, b * ZB * 8:(b + 1) * ZB * 8].rearrange(
            "p (s c j) -> p s c j", s=ZB, c=2, j=4)
        nc.vector.tensor_copy(z, xv)
        ld = None
        # softplus(u + bf0) = ln(1 + exp(u + bf0)) — Softplus has no ACT
        # table set on this toolchain; Exp and Ln share one set.  All Exp
        # ops are emitted before any Ln to avoid table-set ping-pong.
        exs = []
        for i in range(S):
            ex = zp.tile(zsh, f32, tag=f"ex{i}")
            nc.scalar.activation(ex, V[:, 0, i], AF.Exp, bias=col(30 + i))
            exs.append(ex)
        for i in range(S):
            s_v = V[:, 1, i]
            sp = zp.tile(zsh, f32, tag="sp")
            nc.scalar.activation(sp, exs[i], AF.Ln, bias=1.0)
            sc = zp.tile(zsh, f32, tag="sc")
            nc.vector.tensor_scalar_add(sc, sp, 1e-3)
            ldi = zp.tile(zsh, f32, tag="ldi")
            nc.scalar.activation(ldi, sp, AF.Ln, bias=col(33))
            if ld is None:
                ld = ldi
            else:
                ld2 = zp.tile(zsh, f32, tag="ld")
                nc.vector.tensor_tensor(ld2, ld, ldi, OP.add)
                ld = ld2
            z2 = zp.tile(zsh, f32, tag="z")
            nc.vector.tensor_tensor(z2, z, sc, OP.mult)
            sh = zp.tile(zsh, f32, tag="sh")
            nc.vector.tensor_scalar_add(sh, s_v, float(bf[i, 1]))
            z3 = zp.tile(zsh, f32, tag="z")
            nc.vector.tensor_tensor(z3, z2, sh, OP.add)
            z = z3
        zz = zp.tile(zsh, f32, tag="zz")
        nc.vector.tensor_tensor(zz, z, z, OP.mult)
        lp1 = zp.tile(zsh, f32, tag="lp1")
        nc.vector.tensor_scalar(lp1, zz, -0.5, -0.5 * LOG_2PI, OP.mult, OP.add)
        lp = zp.tile(zsh, f32, tag="lp")
        nc.vector.tensor_tensor(lp, lp1, ld, OP.add)

        # ---------- features + output assembly for the ZB supertiles ----------
        for sl in range(ZB):
            s = b * ZB + sl
            outt = io.tile([128, 8 * DOUT], f32, tag="outt")
            nc.scalar.copy(outt[0:1, 0:1], actscr)
            outr = outt.rearrange("p (k c) -> p k c", c=DOUT)
            tso_g = io.tile([3, ST], bf16, tag="tsog")
            nc.sync.dma_start(out=tso_g, in_=tso[:, s * ST:(s + 1) * ST])
            for k in range(8):
                fp = pft.tile([128, 2 * D], f32, tag="fp")
                lhs = tso_g[:, k * 128:(k + 1) * 128]
                nc.tensor.matmul(fp[:, 0:D], lhs, rh_sb[:, 0:D], start=True, stop=True)
                nc.tensor.matmul(fp[:, D:2 * D], lhs, rh_sb[:, D:2 * D],
                                 start=True, stop=True)
                if k % 2 == 0:
                    nc.scalar.copy(outr[:, k, 0:2 * D], fp)
                else:
                    nc.vector.tensor_copy(outr[:, k, 0:2 * D], fp)
            lpv = outt.rearrange("p (c j cc) -> p c j cc", c=2, j=4)[:, :, :, 2 * D]
            nc.vector.tensor_copy(lpv, lp[:, sl])
            ydst = y.rearrange("(s k p) c -> s p k c", p=128, k=8)[s]
            nc.sync.dma_start(out=ydst, in_=outt)


def _build_module(bf):
    nc = bacc.Bacc("TRN2", target_bir_lowering=False, debug=False,
                   enable_asserts=False, num_devices=NCORES)
    y = nc.dram_tensor("y", [N, DOUT], f32, kind="ExternalOutput").ap()
    tso = nc.dram_tensor("tso", [3, N], bf16, kind="ExternalInput").ap()
    cprev = nc.dram_tensor("cprev", [N], f32, kind="ExternalInput").ap()
    resid = nc.dram_tensor("resid", [N], f32, kind="ExternalInput").ap()
    wmm = nc.dram_tensor("wmm", [128, S * NBLK * 128 * 2 + 4 * S + 2 * D], bf16, kind="ExternalInput").ap()
    aux = nc.dram_tensor("aux", [128, 6 + 4 * S * NBLK + S + 1], f32, kind="ExternalInput").ap()
    with tile.TileContext(nc) as tc:
        _body(tc, bf, y, tso, cprev, resid, wmm, aux)
    nc.compile()
    return nc


def _run(inputs, trace=False):
    wp = _prep_weights(inputs)
    bf = np.asarray(inputs["bf"], np.float32)
    nc = _build_module(bf)

    trend = np.asarray(inputs["trend"], np.float32)
    seasonal = np.asarray(inputs["seasonal"], np.float32)
    residual = np.asarray(inputs["residual"], np.float32)
    prev = np.concatenate([np.zeros_like(residual[:, :1]), residual[:, :-1]], axis=1)

    in_maps = []
    for c in range(NCORES):
        sl = slice(c * BP, (c + 1) * BP)
        tso = np.empty((3, N), ml_dtypes.bfloat16)
        tso[0] = trend[sl].reshape(-1).astype(ml_dtypes.bfloat16)
        tso[1] = seasonal[sl].reshape(-1).astype(ml_dtypes.bfloat16)
        tso[2] = 1.0
        in_maps.append({
            "tso": tso,
            "cprev": np.ascontiguousarray(prev[sl].reshape(-1)),
            "resid": np.ascontiguousarray(residual[sl].reshape(-1)),
            "wmm": wp["wmm"], "aux": wp["aux"],
        })

    res = run_bass_kernel_spmd(nc, in_maps, core_ids=list(range(NCORES)),
                               trace=trace)
    out = np.concatenate(
        [r["y"].reshape(BP, T, DOUT) for r in res.results], axis=0)
    return out, res


def kernel(**inputs):
    out, _ = _run(inputs, trace=False)
    return out


# revision 20
# speedup vs baseline: 2.4255x; 1.3438x over previous
"""Trainium2 Bass kernel for nn_ConditionalNFEncoder.

Computes, for inputs trend/seasonal/residual [B, T]:
  feat_trend    = trend[..., None] * Wt[:, 0] + bt        # [B, T, D]
  feat_seasonal = seasonal[..., None] * Ws[:, 0] + bs     # [B, T, D]
  lp            = MADE-flow log-prob of residual given shifted residual
  out           = concat([feat_trend, feat_seasonal, lp[..., None]], -1)

Sharding: pure data parallel over B across 8 NeuronCores (4 rows each).
Inside a core, tokens are processed in "supertiles" of 1024 tokens: the
flow hidden dim (H=64) is packed twice onto the 128 SBUF partitions
(chunk0 tokens on partitions 0:63, chunk1 on 64:127), free dim = 512
tokens.  The two Linear(1, D) features are computed as K=3 matmuls
(trend/seasonal/ones stationary, [Wt|0 / 0|Ws / bt|bs] moving) directly
in token-major layout, copied PSUM->SBUF, and DMA'd out together with
the log-prob column as [128, 8*1025] tiles.
"""

import numpy as np
import ml_dtypes

import concourse.bass as bass
import concourse.bacc as bacc
import concourse.tile as tile
from concourse import mybir
from concourse._compat import with_exitstack
from concourse.bass_utils import run_bass_kernel_spmd

# Problem constants (hardcoded per contract).
B, T, D, H, S, NBLK = 32, 2048, 512, 64, 3, 2
NCORES = 8
BP = B // NCORES            # batch rows per core = 4
N = BP * T                  # tokens per core = 8192
F = 512                     # flow tile free width (tokens per packed chunk)
ST = 2 * F                  # tokens per supertile = 1024
NST = N // ST               # supertiles per core = 8
ZB = 4                      # supertiles per z-chain batch
NCH = N // 128              # 128-token chunks per core = 64
DOUT = 2 * D + 1            # 1025
LOG_2PI = float(np.log(2.0 * np.pi))

f32 = mybir.dt.float32
bf16 = mybir.dt.bfloat16
AF = mybir.ActivationFunctionType
OP = mybir.AluOpType


def _pack2(v):
    """[H] -> [128] duplicated (chunk0 partitions 0:64, chunk1 64:128)."""
    return np.concatenate([v, v]).astype(np.float32)


def _blockdiag2(m):
    """[H, H] -> [128, 128] block-diagonal with two copies of m."""
    z = np.zeros((2 * H, 2 * H), np.float32)
    z[:H, :H] = m
    z[H:, H:] = m
    return z


def _prep_weights(inp):
    """Host-side packing of the tiny flow / feature weights."""
    w1t = np.zeros((128, S * NBLK * 128), np.float32)
    w2t = np.zeros((128, S * NBLK * 128), np.float32)
    cols = np.zeros((128, 6 + 4 * S * NBLK + S + 1), np.float32)
    wft = np.zeros((128, 4 * S), np.float32)
    for i in range(S):
        cols[:, 30 + i] = float(inp["bf"][i, 0])
    cols[:, 33] = 1e-3
    for i in range(S):
        cols[:, 2 * i] = _pack2(inp["Wc0"][i, :, 0])
        cols[:, 2 * i + 1] = _pack2(inp["bc0"][i] + inp["b_init"][i])
        # wft cols for step i: [u_c0, s_c0, u_c1, s_c1]
        wft[:H, 4 * i + 0] = inp["Wf"][i, 0, :]
        wft[:H, 4 * i + 1] = inp["Wf"][i, 1, :]
        wft[H:, 4 * i + 2] = inp["Wf"][i, 0, :]
        wft[H:, 4 * i + 3] = inp["Wf"][i, 1, :]
        for j in range(NBLK):
            q = i * NBLK + j
            w1t[:, q * 128:(q + 1) * 128] = _blockdiag2(inp["W1"][i, j].T)
            w2t[:, q * 128:(q + 1) * 128] = _blockdiag2(inp["W2"][i, j].T)
            cols[:, 6 + 4 * q + 0] = _pack2(inp["b1"][i, j])
            cols[:, 6 + 4 * q + 1] = _pack2(inp["b2"][i, j])
            cols[:, 6 + 4 * q + 2] = _pack2(inp["Wcb"][i, j, :, 0])
            cols[:, 6 + 4 * q + 3] = _pack2(inp["bcb"][i, j])
    rh = np.zeros((3, 2 * D), np.float32)
    rh[0, :D] = inp["Wt"][:, 0]
    rh[1, D:] = inp["Ws"][:, 0]
    rh[2, :D] = inp["bt"]
    rh[2, D:] = inp["bs"]
    # Merge into two tensors so all constants arrive on two DMA-lane sems:
    # wmm (bf16): PE operands [w1t | w2t | wft | rh(padded)] -> [128, 2572]
    # aux (f32):  per-partition scalar columns               -> [128, 34]
    rhp = np.zeros((128, 2 * D), np.float32)
    rhp[:3] = rh
    wmm = np.concatenate([w1t, w2t, wft, rhp], axis=1).astype(ml_dtypes.bfloat16)
    return {"wmm": wmm, "aux": cols, "ncols": cols.shape[1]}


def _bcast2_ap(dram_ap_1d, offset, width):
    """DRAM [2*width] slice as a [2, 64, width] AP: two width-chunks, each
    broadcast over 64 partitions (step-0 middle dim).  Zips element-for-
    element with a [128, width] SBUF tile (partition p = 64*c + lane)."""
    s = dram_ap_1d[offset:offset + width]
    return bass.AP(tensor=s.tensor, offset=s.offset,
                   ap=[[width, 2], [0, 64], [1, width]])


@with_exitstack
def _body(ctx, tc, bf, y, tso, cprev, resid, wmm, aux):
    nc = tc.nc

    const = ctx.enter_context(tc.tile_pool(name="const", bufs=1))
    io = ctx.enter_context(tc.tile_pool(name="io", bufs=2))
    flow = ctx.enter_context(tc.tile_pool(name="flow", bufs=3))
    zp = ctx.enter_context(tc.tile_pool(name="zp", bufs=3))
    pmm = ctx.enter_context(tc.tile_pool(name="pmm", bufs=2, space="PSUM"))
    pzt = ctx.enter_context(tc.tile_pool(name="pzt", bufs=2, space="PSUM"))
    pft = ctx.enter_context(tc.tile_pool(name="pft", bufs=1, space="PSUM"))

    # ---- constants into SBUF (two DMAs -> two lane sems) ----
    NCOLS = 6 + 4 * S * NBLK + S + 1  # 34
    WMMW = S * NBLK * 128 * 2 + 4 * S + 2 * D  # 2572 bf16 cols
    wmm_sb = const.tile([128, WMMW], bf16)
    nc.sync.dma_start(out=wmm_sb, in_=wmm)
    aux_sb = const.tile([128, NCOLS], f32)
    nc.sync.dma_start(out=aux_sb, in_=aux)
    x_sb = const.tile([128, NCH], f32)
    nc.sync.dma_start(out=x_sb, in_=resid.rearrange("(g p) -> p g", p=128))

    w1t_sb = wmm_sb[:, 0:S * NBLK * 128]
    w2t_sb = wmm_sb[:, S * NBLK * 128:2 * S * NBLK * 128]
    wft_sb = wmm_sb[:, 2 * S * NBLK * 128:2 * S * NBLK * 128 + 4 * S]
    rh_sb = wmm_sb[0:3, 2 * S * NBLK * 128 + 4 * S:]

    def col(c):
        return aux_sb[:, c:c + 1]

    # ACT warm-up observer: one single-wait ACT op that makes the ACT
    # engine's vector clock pass the aux DMA lane, so no later ACT
    # instruction (which can encode only ONE sem wait) re-waits it.
    actscr = const.tile([1, 1], f32)
    nc.scalar.copy(actscr, aux_sb[0:1, 0:1])

    for b in range(NST // ZB):
        zt_sb = zp.tile([128, ZB * 4 * S * 4], f32, tag="ztsb")  # [128, 96]

        # ---------- flow, software-pipelined over pairs of supertiles ----
        # Two independent supertile streams are interleaved at the
        # instruction level so each engine's FIFO always has a ready
        # instruction while the other stream waits on a cross-engine dep.
        for half in range(ZB // 2):
            ks = [2 * half, 2 * half + 1]          # local supertile indices
            cb, cb2, h = [None, None], [None, None], [None, None]
            # one PSUM bank holds both supertiles' (uscale, shift) columns
            zt_ps = pzt.tile([128, 2 * 4 * S * 4], f32, tag="ztps")  # [128, 96]
            for k, sl in enumerate(ks):
                cb[k] = flow.tile([128, F], bf16, tag=f"cb{k}", name=f"cb{k}")
                nc.gpsimd.dma_start(out=cb[k],
                                    in_=_bcast2_ap(cprev, (b * ZB + sl) * ST, F))
            for k in range(2):
                # DVE-owned copy for the ACT sigmoid: keeps every consumer
                # of a tile on one engine so no instruction needs >1 wait
                # (the ACT instruction encoding supports only one).
                cb2[k] = flow.tile([128, F], bf16, tag=f"cb2{k}", name=f"cb2{k}")
                nc.vector.tensor_copy(cb2[k], cb[k])
            for i in range(S):
                for k in range(2):
                    h[k] = flow.tile([128, F], bf16, tag=f"h{k}", name=f"h{k}")
                    nc.vector.tensor_scalar(h[k], cb[k], col(2 * i),
                                            col(2 * i + 1), OP.mult, OP.add)
                for j in range(NBLK):
                    q = i * NBLK + j
                    r, p1, r1, p2, sg, t2, m = ({}, {}, {}, {}, {}, {}, {})
                    for k in range(2):
                        r[k] = flow.tile([128, F], bf16, tag=f"r{k}", name=f"r{k}")
                        nc.vector.tensor_scalar_max(r[k], h[k], 0.0)
                    for k in range(2):
                        p1[k] = pmm.tile([128, F], f32, tag=f"pmm{k}", name=f"p1_{k}")
                        nc.tensor.matmul(p1[k], w1t_sb[:, q * 128:(q + 1) * 128],
                                         r[k], start=True, stop=True)
                    for k in range(2):
                        r1[k] = flow.tile([128, F], bf16, tag=f"r1{k}", name=f"r1_{k}")
                        nc.scalar.activation(r1[k], p1[k], AF.Relu,
                                             bias=col(6 + 4 * q + 0))
                    for k in range(2):
                        p2[k] = pmm.tile([128, F], f32, tag=f"pmm{k}", name=f"p2_{k}")
                        nc.tensor.matmul(p2[k], w2t_sb[:, q * 128:(q + 1) * 128],
                                         r1[k], start=True, stop=True)
                    for k in range(2):
                        sg[k] = flow.tile([128, F], bf16, tag=f"sg{k}", name=f"sg{k}")
                        nc.scalar.activation(sg[k], cb2[k], AF.Sigmoid,
                                             bias=col(6 + 4 * q + 3),
                                             scale=col(6 + 4 * q + 2))
                    for k in range(2):
                        t2[k] = flow.tile([128, F], bf16, tag=f"t2{k}", name=f"t2_{k}")
                        nc.vector.tensor_scalar_add(t2[k], p2[k], col(6 + 4 * q + 1))
                    for k in range(2):
                        m[k] = flow.tile([128, F], bf16, tag=f"m{k}", name=f"m{k}")
                        nc.vector.tensor_tensor(m[k], t2[k], sg[k], OP.mult)
                    for k in range(2):
                        h2 = flow.tile([128, F], bf16, tag=f"h{k}")
                        nc.vector.tensor_tensor(h2, h[k], m[k], OP.add)
                        h[k] = h2
                r2 = {}
                for k in range(2):
                    r2[k] = flow.tile([128, F], bf16, tag=f"r{k}", name=f"r2_{k}")
                    nc.vector.tensor_scalar_max(r2[k], h[k], 0.0)
                # transpose (uscale, shift) to token-major via tiny matmuls:
                # out[p_tok, 4] = r2[:, j2-chunk].T @ wft_i
                for k in range(2):
                    for j2 in range(4):
                        c0 = k * 48 + 4 * (S * j2 + i)
                        nc.tensor.matmul(zt_ps[:, c0:c0 + 4],
                                         r2[k][:, 128 * j2:128 * (j2 + 1)],
                                         wft_sb[:, 4 * i:4 * i + 4],
                                         start=True, stop=True)
            nc.vector.tensor_copy(zt_sb[:, half * 96:(half + 1) * 96], zt_ps)

        # ---------- z-chain for this batch (token-major, [128, ZB, 2, 4]) ----------
        # zt_sb col = sl*48 + j2*12 + i*4 + c*2 + t
        V = zt_sb.rearrange("p (s j i c t) -> p t i s c j", s=ZB, j=4, i=S, c=2, t=2)
        zsh = [128, ZB, 2, 4]
        z = zp.tile(zsh, f32, tag="z")
        xv = x_sb[:, b * ZB * 8:(b + 1) * ZB * 8].rearrange(
            "p (s c j) -> p s c j", s=ZB, c=2, j=4)
        nc.vector.tensor_copy(z, xv)
        ld = None
        # softplus(u + bf0) = ln(1 + exp(u + bf0)) — Softplus has no ACT
        # table set on this toolchain; Exp and Ln share one set.  All Exp
        # ops are emitted before any Ln to avoid table-set ping-pong.
        exs = []
        for i in range(S):
            ex = zp.tile(zsh, f32, tag=f"ex{i}")
            nc.scalar.activation(ex, V[:, 0, i], AF.Exp, bias=col(30 + i))
            exs.append(ex)
        for i in range(S):
            s_v = V[:, 1, i]
            sp = zp.tile(zsh, f32, tag="sp")
            nc.scalar.activation(sp, exs[i], AF.Ln, bias=1.0)
            sc = zp.tile(zsh, f32, tag="sc")
            nc.vector.tensor_scalar_add(sc, sp, 1e-3)
            ldi = zp.tile(zsh, f32, tag="ldi")
            nc.scalar.activation(ldi, sp, AF.Ln, bias=col(33))
            if ld is None:
                ld = ldi
            else:
                ld2 = zp.tile(zsh, f32, tag="ld")
                nc.vector.tensor_tensor(ld2, ld, ldi, OP.add)
                ld = ld2
            z2 = zp.tile(zsh, f32, tag="z")
            nc.vector.tensor_tensor(z2, z, sc, OP.mult)
            sh = zp.tile(zsh, f32, tag="sh")
            nc.vector.tensor_scalar_add(sh, s_v, float(bf[i, 1]))
            z3 = zp.tile(zsh, f32, tag="z")
            nc.vector.tensor_tensor(z3, z2, sh, OP.add)
            z = z3
        zz = zp.tile(zsh, f32, tag="zz")
        nc.vector.tensor_tensor(zz, z, z, OP.mult)
        lp1 = zp.tile(zsh, f32, tag="lp1")
        nc.vector.tensor_scalar(lp1, zz, -0.5, -0.5 * LOG_2PI, OP.mult, OP.add)
        lp = zp.tile(zsh, f32, tag="lp")
        nc.vector.tensor_tensor(lp, lp1, ld, OP.add)

        # ---------- features + output assembly for the ZB supertiles ----------
        for sl in range(ZB):
            s = b * ZB + sl
            outt = io.tile([128, 8 * DOUT], f32, tag="outt")
            nc.scalar.copy(outt[0:1, 0:1], actscr)
            outr = outt.rearrange("p (k c) -> p k c", c=DOUT)
            tso_g = io.tile([3, ST], bf16, tag="tsog")
            nc.sync.dma_start(out=tso_g, in_=tso[:, s * ST:(s + 1) * ST])
            for k in range(8):
                fp = pft.tile([128, 2 * D], f32, tag="fp")
                lhs = tso_g[:, k * 128:(k + 1) * 128]
                nc.tensor.matmul(fp[:, 0:D], lhs, rh_sb[:, 0:D], start=True, stop=True)
                nc.tensor.matmul(fp[:, D:2 * D], lhs, rh_sb[:, D:2 * D],
                                 start=True, stop=True)
                if k % 2 == 0:
                    nc.scalar.copy(outr[:, k, 0:2 * D], fp)
                else:
                    nc.vector.tensor_copy(outr[:, k, 0:2 * D], fp)
            lpv = outt.rearrange("p (c j cc) -> p c j cc", c=2, j=4)[:, :, :, 2 * D]
            nc.vector.tensor_copy(lpv, lp[:, sl])
            ydst = y.rearrange("(s k p) c -> s p k c", p=128, k=8)[s]
            nc.sync.dma_start(out=ydst, in_=outt)


def _build_module(bf):
    nc = bacc.Bacc("TRN2", target_bir_lowering=False, debug=False,
                   enable_asserts=False, num_devices=NCORES)
    y = nc.dram_tensor("y", [N, DOUT], f32, kind="ExternalOutput").ap()
    tso = nc.dram_tensor("tso", [3, N], bf16, kind="ExternalInput").ap()
    cprev = nc.dram_tensor("cprev", [N], f32, kind="ExternalInput").ap()
    resid = nc.dram_tensor("resid", [N], f32, kind="ExternalInput").ap()
    wmm = nc.dram_tensor("wmm", [128, S * NBLK * 128 * 2 + 4 * S + 2 * D], bf16, kind="ExternalInput").ap()
    aux = nc.dram_tensor("aux", [128, 6 + 4 * S * NBLK + S + 1], f32, kind="ExternalInput").ap()
    with tile.TileContext(nc) as tc:
        _body(tc, bf, y, tso, cprev, resid, wmm, aux)
    nc.compile()
    return nc


def _run(inputs, trace=False):
    wp = _prep_weights(inputs)
    bf = np.asarray(inputs["bf"], np.float32)
    nc = _build_module(bf)

    trend = np.asarray(inputs["trend"], np.float32)
    seasonal = np.asarray(inputs["seasonal"], np.float32)
    residual = np.asarray(inputs["residual"], np.float32)
    prev = np.concatenate([np.zeros_like(residual[:, :1]), residual[:, :-1]], axis=1)

    in_maps = []
    for c in range(NCORES):
        sl = slice(c * BP, (c + 1) * BP)
        tso = np.empty((3, N), ml_dtypes.bfloat16)
        tso[0] = trend[sl].reshape(-1).astype(ml_dtypes.bfloat16)
        tso[1] = seasonal[sl].reshape(-1).astype(ml_dtypes.bfloat16)
        tso[2] = 1.0
        in_maps.append({
            "tso": tso,
            "cprev": np.ascontiguousarray(prev[sl].reshape(-1)),
            "resid": np.ascontiguousarray(residual[sl].reshape(-1)),
            "wmm": wp["wmm"], "aux": wp["aux"],
        })

    res = run_bass_kernel_spmd(nc, in_maps, core_ids=list(range(NCORES)),
                               trace=trace)
    out = np.concatenate(
        [r["y"].reshape(BP, T, DOUT) for r in res.results], axis=0)
    return out, res


def kernel(**inputs):
    out, _ = _run(inputs, trace=False)
    return out


# revision 21
# speedup vs baseline: 2.6244x; 1.0820x over previous
"""Trainium2 Bass kernel for nn_ConditionalNFEncoder.

Computes, for inputs trend/seasonal/residual [B, T]:
  feat_trend    = trend[..., None] * Wt[:, 0] + bt        # [B, T, D]
  feat_seasonal = seasonal[..., None] * Ws[:, 0] + bs     # [B, T, D]
  lp            = MADE-flow log-prob of residual given shifted residual
  out           = concat([feat_trend, feat_seasonal, lp[..., None]], -1)

Sharding: pure data parallel over B across 8 NeuronCores (4 rows each).
Inside a core, tokens are processed in "supertiles" of 1024 tokens: the
flow hidden dim (H=64) is packed twice onto the 128 SBUF partitions
(chunk0 tokens on partitions 0:63, chunk1 on 64:127), free dim = 512
tokens.  The two Linear(1, D) features are computed as K=3 matmuls
(trend/seasonal/ones stationary, [Wt|0 / 0|Ws / bt|bs] moving) directly
in token-major layout, copied PSUM->SBUF, and DMA'd out together with
the log-prob column as [128, 8*1025] tiles.
"""

import numpy as np
import ml_dtypes

import concourse.bass as bass
import concourse.bacc as bacc
import concourse.tile as tile
from concourse import mybir
from concourse._compat import with_exitstack
from concourse.bass_utils import run_bass_kernel_spmd

# Problem constants (hardcoded per contract).
B, T, D, H, S, NBLK = 32, 2048, 512, 64, 3, 2
NCORES = 8
BP = B // NCORES            # batch rows per core = 4
N = BP * T                  # tokens per core = 8192
F = 512                     # flow tile free width (tokens per packed chunk)
ST = 2 * F                  # tokens per supertile = 1024
NST = N // ST               # supertiles per core = 8
ZB = 4                      # supertiles per z-chain batch
NCH = N // 128              # 128-token chunks per core = 64
DOUT = 2 * D + 1            # 1025
LOG_2PI = float(np.log(2.0 * np.pi))

f32 = mybir.dt.float32
bf16 = mybir.dt.bfloat16
AF = mybir.ActivationFunctionType
OP = mybir.AluOpType


def _pack2(v):
    """[H] -> [128] duplicated (chunk0 partitions 0:64, chunk1 64:128)."""
    return np.concatenate([v, v]).astype(np.float32)


def _blockdiag2(m):
    """[H, H] -> [128, 128] block-diagonal with two copies of m."""
    z = np.zeros((2 * H, 2 * H), np.float32)
    z[:H, :H] = m
    z[H:, H:] = m
    return z


def _prep_weights(inp):
    """Host-side packing of the tiny flow / feature weights."""
    w1t = np.zeros((128, S * NBLK * 128), np.float32)
    w2t = np.zeros((128, S * NBLK * 128), np.float32)
    cols = np.zeros((128, 6 + 4 * S * NBLK + S + 1), np.float32)
    wft = np.zeros((128, 4 * S), np.float32)
    for i in range(S):
        cols[:, 30 + i] = float(inp["bf"][i, 0])
    cols[:, 33] = 1e-3
    for i in range(S):
        cols[:, 2 * i] = _pack2(inp["Wc0"][i, :, 0])
        cols[:, 2 * i + 1] = _pack2(inp["bc0"][i] + inp["b_init"][i])
        # wft cols for step i: [u_c0, s_c0, u_c1, s_c1]
        wft[:H, 4 * i + 0] = inp["Wf"][i, 0, :]
        wft[:H, 4 * i + 1] = inp["Wf"][i, 1, :]
        wft[H:, 4 * i + 2] = inp["Wf"][i, 0, :]
        wft[H:, 4 * i + 3] = inp["Wf"][i, 1, :]
        for j in range(NBLK):
            q = i * NBLK + j
            w1t[:, q * 128:(q + 1) * 128] = _blockdiag2(inp["W1"][i, j].T)
            w2t[:, q * 128:(q + 1) * 128] = _blockdiag2(inp["W2"][i, j].T)
            cols[:, 6 + 4 * q + 0] = _pack2(inp["b1"][i, j])
            cols[:, 6 + 4 * q + 1] = _pack2(inp["b2"][i, j])
            cols[:, 6 + 4 * q + 2] = _pack2(inp["Wcb"][i, j, :, 0])
            cols[:, 6 + 4 * q + 3] = _pack2(inp["bcb"][i, j])
    rh = np.zeros((3, 2 * D), np.float32)
    rh[0, :D] = inp["Wt"][:, 0]
    rh[1, D:] = inp["Ws"][:, 0]
    rh[2, :D] = inp["bt"]
    rh[2, D:] = inp["bs"]
    # Merge into two tensors so all constants arrive on two DMA-lane sems:
    # wmm (bf16): PE operands [w1t | w2t | wft | rh(padded)] -> [128, 2572]
    # aux (f32):  per-partition scalar columns               -> [128, 34]
    rhp = np.zeros((128, 2 * D), np.float32)
    rhp[:3] = rh
    wmm = np.concatenate([w1t, w2t, wft, rhp], axis=1).astype(ml_dtypes.bfloat16)
    return {"wmm": wmm, "aux": cols, "ncols": cols.shape[1]}


def _bcast2_ap(dram_ap_1d, offset, width):
    """DRAM [2*width] slice as a [2, 64, width] AP: two width-chunks, each
    broadcast over 64 partitions (step-0 middle dim).  Zips element-for-
    element with a [128, width] SBUF tile (partition p = 64*c + lane)."""
    s = dram_ap_1d[offset:offset + width]
    return bass.AP(tensor=s.tensor, offset=s.offset,
                   ap=[[width, 2], [0, 64], [1, width]])


@with_exitstack
def _body(ctx, tc, bf, y, tso, cprev, resid, wmm, aux):
    nc = tc.nc

    const = ctx.enter_context(tc.tile_pool(name="const", bufs=1))
    io = ctx.enter_context(tc.tile_pool(name="io", bufs=3))
    flow = ctx.enter_context(tc.tile_pool(name="flow", bufs=3))
    zp = ctx.enter_context(tc.tile_pool(name="zp", bufs=3))
    pmm = ctx.enter_context(tc.tile_pool(name="pmm", bufs=2, space="PSUM"))
    pzt = ctx.enter_context(tc.tile_pool(name="pzt", bufs=2, space="PSUM"))
    pft = ctx.enter_context(tc.tile_pool(name="pft", bufs=1, space="PSUM"))

    # ---- constants into SBUF (two DMAs -> two lane sems) ----
    NCOLS = 6 + 4 * S * NBLK + S + 1  # 34
    WMMW = S * NBLK * 128 * 2 + 4 * S + 2 * D  # 2572 bf16 cols
    wmm_sb = const.tile([128, WMMW], bf16)
    nc.sync.dma_start(out=wmm_sb, in_=wmm)
    aux_sb = const.tile([128, NCOLS], f32)
    nc.sync.dma_start(out=aux_sb, in_=aux)
    x_sb = const.tile([128, NCH], f32)
    nc.sync.dma_start(out=x_sb, in_=resid.rearrange("(g p) -> p g", p=128))

    w1t_sb = wmm_sb[:, 0:S * NBLK * 128]
    w2t_sb = wmm_sb[:, S * NBLK * 128:2 * S * NBLK * 128]
    wft_sb = wmm_sb[:, 2 * S * NBLK * 128:2 * S * NBLK * 128 + 4 * S]
    rh_sb = wmm_sb[0:3, 2 * S * NBLK * 128 + 4 * S:]

    def col(c):
        return aux_sb[:, c:c + 1]

    # ACT warm-up observer: one single-wait ACT op that makes the ACT
    # engine's vector clock pass the aux DMA lane, so no later ACT
    # instruction (which can encode only ONE sem wait) re-waits it.
    actscr = const.tile([1, 1], f32)
    nc.scalar.copy(actscr, aux_sb[0:1, 0:1])

    for b in range(NST // ZB):
        zt_sb = zp.tile([128, ZB * 4 * S * 4], f32, tag="ztsb")  # [128, 96]

        # ---------- flow, software-pipelined over pairs of supertiles ----
        # Two independent supertile streams are interleaved at the
        # instruction level so each engine's FIFO always has a ready
        # instruction while the other stream waits on a cross-engine dep.
        for half in range(ZB // 2):
            ks = [2 * half, 2 * half + 1]          # local supertile indices
            cb, cb2, h = [None, None], [None, None], [None, None]
            # one PSUM bank holds both supertiles' (uscale, shift) columns
            zt_ps = pzt.tile([128, 2 * 4 * S * 4], f32, tag="ztps")  # [128, 96]
            for k, sl in enumerate(ks):
                cb[k] = flow.tile([128, F], bf16, tag=f"cb{k}", name=f"cb{k}")
                nc.gpsimd.dma_start(out=cb[k],
                                    in_=_bcast2_ap(cprev, (b * ZB + sl) * ST, F))
            for k in range(2):
                # DVE-owned copy for the ACT sigmoid: keeps every consumer
                # of a tile on one engine so no instruction needs >1 wait
                # (the ACT instruction encoding supports only one).
                cb2[k] = flow.tile([128, F], bf16, tag=f"cb2{k}", name=f"cb2{k}")
                nc.vector.tensor_copy(cb2[k], cb[k])
            for i in range(S):
                for k in range(2):
                    h[k] = flow.tile([128, F], bf16, tag=f"h{k}", name=f"h{k}")
                    nc.vector.tensor_scalar(h[k], cb[k], col(2 * i),
                                            col(2 * i + 1), OP.mult, OP.add)
                for j in range(NBLK):
                    q = i * NBLK + j
                    r, p1, r1, p2, sg, t2, m = ({}, {}, {}, {}, {}, {}, {})
                    for k in range(2):
                        r[k] = flow.tile([128, F], bf16, tag=f"r{k}", name=f"r{k}")
                        nc.vector.tensor_scalar_max(r[k], h[k], 0.0)
                    for k in range(2):
                        p1[k] = pmm.tile([128, F], f32, tag=f"pmm{k}", name=f"p1_{k}")
                        nc.tensor.matmul(p1[k], w1t_sb[:, q * 128:(q + 1) * 128],
                                         r[k], start=True, stop=True)
                    for k in range(2):
                        r1[k] = flow.tile([128, F], bf16, tag=f"r1{k}", name=f"r1_{k}")
                        nc.scalar.activation(r1[k], p1[k], AF.Relu,
                                             bias=col(6 + 4 * q + 0))
                    for k in range(2):
                        p2[k] = pmm.tile([128, F], f32, tag=f"pmm{k}", name=f"p2_{k}")
                        nc.tensor.matmul(p2[k], w2t_sb[:, q * 128:(q + 1) * 128],
                                         r1[k], start=True, stop=True)
                    for k in range(2):
                        sg[k] = flow.tile([128, F], bf16, tag=f"sg{k}", name=f"sg{k}")
                        nc.scalar.activation(sg[k], cb2[k], AF.Sigmoid,
                                             bias=col(6 + 4 * q + 3),
                                             scale=col(6 + 4 * q + 2))
                    for k in range(2):
                        t2[k] = flow.tile([128, F], bf16, tag=f"t2{k}", name=f"t2_{k}")
                        nc.scalar.activation(t2[k], p2[k], AF.Identity,
                                             bias=col(6 + 4 * q + 1))
                    for k in range(2):
                        m[k] = flow.tile([128, F], bf16, tag=f"m{k}", name=f"m{k}")
                        nc.vector.tensor_tensor(m[k], t2[k], sg[k], OP.mult)
                    for k in range(2):
                        h2 = flow.tile([128, F], bf16, tag=f"h{k}")
                        nc.vector.tensor_tensor(h2, h[k], m[k], OP.add)
                        h[k] = h2
                r2 = {}
                for k in range(2):
                    r2[k] = flow.tile([128, F], bf16, tag=f"r{k}", name=f"r2_{k}")
                    nc.vector.tensor_scalar_max(r2[k], h[k], 0.0)
                # transpose (uscale, shift) to token-major via tiny matmuls:
                # out[p_tok, 4] = r2[:, j2-chunk].T @ wft_i
                for k in range(2):
                    for j2 in range(4):
                        c0 = k * 48 + 4 * (S * j2 + i)
                        nc.tensor.matmul(zt_ps[:, c0:c0 + 4],
                                         r2[k][:, 128 * j2:128 * (j2 + 1)],
                                         wft_sb[:, 4 * i:4 * i + 4],
                                         start=True, stop=True)
            nc.vector.tensor_copy(zt_sb[:, half * 96:(half + 1) * 96], zt_ps)

        # ---------- z-chain for this batch (token-major, [128, ZB, 2, 4]) ----------
        # zt_sb col = sl*48 + j2*12 + i*4 + c*2 + t
        V = zt_sb.rearrange("p (s j i c t) -> p t i s c j", s=ZB, j=4, i=S, c=2, t=2)
        zsh = [128, ZB, 2, 4]
        z = zp.tile(zsh, f32, tag="z")
        xv = x_sb[:, b * ZB * 8:(b + 1) * ZB * 8].rearrange(
            "p (s c j) -> p s c j", s=ZB, c=2, j=4)
        nc.vector.tensor_copy(z, xv)
        ld = None
        # softplus(u + bf0) = ln(1 + exp(u + bf0)) — Softplus has no ACT
        # table set on this toolchain; Exp and Ln share one set.  All Exp
        # ops are emitted before any Ln to avoid table-set ping-pong.
        exs = []
        for i in range(S):
            ex = zp.tile(zsh, f32, tag=f"ex{i}")
            nc.scalar.activation(ex, V[:, 0, i], AF.Exp, bias=col(30 + i))
            exs.append(ex)
        for i in range(S):
            s_v = V[:, 1, i]
            sp = zp.tile(zsh, f32, tag="sp")
            nc.scalar.activation(sp, exs[i], AF.Ln, bias=1.0)
            sc = zp.tile(zsh, f32, tag="sc")
            nc.vector.tensor_scalar_add(sc, sp, 1e-3)
            ldi = zp.tile(zsh, f32, tag="ldi")
            nc.scalar.activation(ldi, sp, AF.Ln, bias=col(33))
            if ld is None:
                ld = ldi
            else:
                ld2 = zp.tile(zsh, f32, tag="ld")
                nc.vector.tensor_tensor(ld2, ld, ldi, OP.add)
                ld = ld2
            z2 = zp.tile(zsh, f32, tag="z")
            nc.vector.tensor_tensor(z2, z, sc, OP.mult)
            sh = zp.tile(zsh, f32, tag="sh")
            nc.vector.tensor_scalar_add(sh, s_v, float(bf[i, 1]))
            z3 = zp.tile(zsh, f32, tag="z")
            nc.vector.tensor_tensor(z3, z2, sh, OP.add)
            z = z3
        zz = zp.tile(zsh, f32, tag="zz")
        nc.vector.tensor_tensor(zz, z, z, OP.mult)
        lp1 = zp.tile(zsh, f32, tag="lp1")
        nc.vector.tensor_scalar(lp1, zz, -0.5, -0.5 * LOG_2PI, OP.mult, OP.add)
        lp = zp.tile(zsh, f32, tag="lp")
        nc.vector.tensor_tensor(lp, lp1, ld, OP.add)

        # ---------- features + output assembly for the ZB supertiles ----------
        for sl in range(ZB):
            s = b * ZB + sl
            outt = io.tile([128, 8 * DOUT], bf16, tag="outt")
            outr = outt.rearrange("p (k c) -> p k c", c=DOUT)
            tso_g = io.tile([3, ST], bf16, tag="tsog")
            nc.sync.dma_start(out=tso_g, in_=tso[:, s * ST:(s + 1) * ST])
            for k in range(8):
                fp = pft.tile([128, 2 * D], f32, tag="fp")
                lhs = tso_g[:, k * 128:(k + 1) * 128]
                nc.tensor.matmul(fp[:, 0:D], lhs, rh_sb[:, 0:D], start=True, stop=True)
                nc.tensor.matmul(fp[:, D:2 * D], lhs, rh_sb[:, D:2 * D],
                                 start=True, stop=True)
                nc.vector.tensor_copy(outr[:, k, 0:2 * D], fp)
            lpv = outt.rearrange("p (c j cc) -> p c j cc", c=2, j=4)[:, :, :, 2 * D]
            nc.vector.tensor_copy(lpv, lp[:, sl])
            ydst = y.rearrange("(s k p) c -> s p k c", p=128, k=8)[s]
            # SWDGE DMA casts bf16 -> f32 on the way out
            nc.gpsimd.dma_start(out=ydst, in_=outt)


def _build_module(bf):
    nc = bacc.Bacc("TRN2", target_bir_lowering=False, debug=False,
                   enable_asserts=False, num_devices=NCORES)
    y = nc.dram_tensor("y", [N, DOUT], f32, kind="ExternalOutput").ap()
    tso = nc.dram_tensor("tso", [3, N], bf16, kind="ExternalInput").ap()
    cprev = nc.dram_tensor("cprev", [N], f32, kind="ExternalInput").ap()
    resid = nc.dram_tensor("resid", [N], f32, kind="ExternalInput").ap()
    wmm = nc.dram_tensor("wmm", [128, S * NBLK * 128 * 2 + 4 * S + 2 * D], bf16, kind="ExternalInput").ap()
    aux = nc.dram_tensor("aux", [128, 6 + 4 * S * NBLK + S + 1], f32, kind="ExternalInput").ap()
    with tile.TileContext(nc) as tc:
        _body(tc, bf, y, tso, cprev, resid, wmm, aux)
    nc.compile()
    return nc


def _run(inputs, trace=False):
    wp = _prep_weights(inputs)
    bf = np.asarray(inputs["bf"], np.float32)
    nc = _build_module(bf)

    trend = np.asarray(inputs["trend"], np.float32)
    seasonal = np.asarray(inputs["seasonal"], np.float32)
    residual = np.asarray(inputs["residual"], np.float32)
    prev = np.concatenate([np.zeros_like(residual[:, :1]), residual[:, :-1]], axis=1)

    in_maps = []
    for c in range(NCORES):
        sl = slice(c * BP, (c + 1) * BP)
        tso = np.empty((3, N), ml_dtypes.bfloat16)
        tso[0] = trend[sl].reshape(-1).astype(ml_dtypes.bfloat16)
        tso[1] = seasonal[sl].reshape(-1).astype(ml_dtypes.bfloat16)
        tso[2] = 1.0
        in_maps.append({
            "tso": tso,
            "cprev": np.ascontiguousarray(prev[sl].reshape(-1)),
            "resid": np.ascontiguousarray(residual[sl].reshape(-1)),
            "wmm": wp["wmm"], "aux": wp["aux"],
        })

    res = run_bass_kernel_spmd(nc, in_maps, core_ids=list(range(NCORES)),
                               trace=trace)
    out = np.concatenate(
        [r["y"].reshape(BP, T, DOUT) for r in res.results], axis=0)
    return out, res


def kernel(**inputs):
    out, _ = _run(inputs, trace=False)
    return out
